# revision 6
# baseline (speedup 1.0000x reference)
"""DropEdge GCN (3-layer, inference) on 8 Trainium2 NeuronCores.

Strategy: partition nodes across the 8 cores by destination (graph parallel).
Per layer, each core builds its slice of the fp16 "message table"
T = dinv * (h @ (W_gcn * g_bn)) (BN scale folded into the weights), two
AllGathers replicate T into every core's DRAM, then dma_gather pulls the
source rows for 128-edge chunks, a one-hot*weight matrix S (built on the
vector engine from iota==dst_local) turns the segment-sum into PSUM-
accumulating matmuls S^T @ M per 128-node destination block. The self-loop
term, the folded BN bias, and the per-node dinv scaling ride the table-build
matmuls / the PSUM-evicting activation op.
"""
import sys
import types
import numpy as np

N = 50000
E = 1_000_000
F_IN = 256
UNITS = 128
NCLS = 40
EPS = 1e-3

NCORES = 8
NPC = 6272            # padded nodes per core
NBT = NCORES * NPC    # 50176
NBLK = 49             # 128-node blocks per core
HALF = 3136           # phase split within a core's node range
FW = 128              # uniform table width (layer-3 output padded 40->128)
BATCH = 64            # chunks per dma_gather call (64*128 = 8192 rows)


def _pad_cols(a, width):
    out = np.zeros((a.shape[0], width), np.float32)
    out[:, : a.shape[1]] = a
    return out


def _host_prep(inputs):
    x = np.asarray(inputs["x"], np.float32)
    ei = np.asarray(inputs["edge_index"])
    w = np.asarray(inputs["edge_weight"], np.float32)
    src = ei[0].astype(np.int64)
    dst = ei[1].astype(np.int64)

    counts = np.bincount(dst, minlength=NBT)
    assert counts[:N].min() > 0, "degree-0 real node: unsupported fast path"
    dpad = int(-(-int(counts.max()) // 4) * 4)

    # weights grouped by dst node (padded CSR) for the on-device degree reduce
    order0 = np.argsort(dst, kind="stable")
    ofs = np.zeros(NBT + 1, np.int64)
    ofs[1:] = np.cumsum(counts)
    slot = np.arange(E) - ofs[dst[order0]]
    wbd_full = np.zeros((NBT, dpad), np.float32)
    wbd_full[dst[order0], slot] = w[order0]
    wbd = np.ascontiguousarray(
        wbd_full.reshape(NCORES, NBLK, 128, dpad).transpose(0, 2, 1, 3)
    )  # [NC, 128, NBLK, dpad]

    # edge groups by (dst core, dst block, src half)
    core_e = dst // NPC
    blk_e = (dst % NPC) // 128
    dloc = (dst % 128).astype(np.float32)
    hsrc = (src % NPC) // HALF
    srow = ((src // NPC) * HALF + (src % NPC) % HALF).astype(np.int64)
    assert srow.max() < NCORES * HALF

    key = (core_e * NBLK + blk_e) * 2 + hsrc
    order = np.argsort(key, kind="stable")
    gcnt = np.bincount(key, minlength=NCORES * NBLK * 2).reshape(NCORES, NBLK, 2)
    gofs = np.zeros(NCORES * NBLK * 2 + 1, np.int64)
    gofs[1:] = np.cumsum(gcnt.reshape(-1))
    C = np.maximum(1, -(-gcnt // 128)).max(axis=0)  # [NBLK, 2] shared chunk counts

    nch = [int(C[:, h].sum()) for h in (0, 1)]
    nbat = [-(-nch[h] // BATCH) for h in (0, 1)]
    nslot = [nbat[h] * BATCH for h in (0, 1)]

    idx_arr = [np.zeros((NCORES, nslot[h] * 128), np.int64) for h in (0, 1)]
    w_arr = [np.zeros((NCORES, nslot[h] * 128), np.float32) for h in (0, 1)]
    d_arr = [np.zeros((NCORES, nslot[h] * 128), np.float32) for h in (0, 1)]
    for c in range(NCORES):
        for h in (0, 1):
            pos = 0
            for j in range(NBLK):
                g = (c * NBLK + j) * 2 + h
                k = gcnt[c, j, h]
                es = order[gofs[g] : gofs[g] + k]
                base = pos * 128
                idx_arr[h][c, base : base + k] = srow[es]
                w_arr[h][c, base : base + k] = w[es]
                d_arr[h][c, base : base + k] = dloc[es]
                pos += int(C[j, h])

    # device layouts
    idx_dev, w_dev, d_dev = [], [], []
    for h in (0, 1):
        ia = np.zeros((NCORES, nbat[h], 32, 512), np.int16)
        for c in range(NCORES):
            fl = idx_arr[h][c].astype(np.int16).reshape(nbat[h], BATCH * 128)
            wrapped = fl.reshape(nbat[h], 512, 16).transpose(0, 2, 1)  # [nb,16,512]
            ia[c, :, 0:16] = wrapped
            ia[c, :, 16:32] = wrapped
        idx_dev.append(ia)
        w_dev.append(
            np.ascontiguousarray(
                w_arr[h].reshape(NCORES, nslot[h], 128).transpose(0, 2, 1)
            )
        )
        d_dev.append(
            np.ascontiguousarray(
                d_arr[h].reshape(NCORES, nslot[h], 128).transpose(0, 2, 1)
            )
        )

    # node features, transposed + padded, per-core column slice
    xt_full = np.zeros((F_IN, NBT), np.float32)
    xt_full[:, :N] = x.T
    xt = np.ascontiguousarray(xt_full.reshape(F_IN, NCORES, NPC).transpose(1, 0, 2))

    # weights (pad layer 3 to 128 wide)
    wg = [
        np.asarray(inputs["w_gcn1"], np.float32),
        np.asarray(inputs["w_gcn2"], np.float32),
        _pad_cols(np.asarray(inputs["w_gcn3"], np.float32), FW),
    ]
    ws = [
        np.asarray(inputs["w_self1"], np.float32),
        np.asarray(inputs["w_self2"], np.float32),
        _pad_cols(np.asarray(inputs["w_self3"], np.float32), FW),
    ]
    bn = []
    for li in (1, 2, 3):
        row = np.zeros((1, 5 * FW), np.float32)
        fo = UNITS if li < 3 else NCLS
        row[0, 0 * FW : 0 * FW + fo] = np.asarray(inputs[f"b{li}"], np.float32)
        row[0, 1 * FW : 1 * FW + fo] = np.asarray(inputs[f"gamma{li}"], np.float32)
        row[0, 2 * FW : 2 * FW + fo] = np.asarray(inputs[f"beta{li}"], np.float32)
        row[0, 3 * FW : 3 * FW + fo] = np.asarray(inputs[f"mean{li}"], np.float32)
        row[0, 4 * FW : 4 * FW + fo] = 1.0
        row[0, 4 * FW : 4 * FW + fo] = np.asarray(inputs[f"var{li}"], np.float32)
        row[0, 4 * FW + fo : 5 * FW] = 1.0  # padded var=1 avoids rsqrt(eps) blowup
        bn.append(row)

    iota16 = np.tile(np.arange(128, dtype=np.float16)[None, :], (128, 1))
    ones_row = np.ones((1, 128), np.float32)

    meta = dict(dpad=dpad, C=C, nbat_a=nbat[0], nbat_b=nbat[1],
                nslot_a=nslot[0], nslot_b=nslot[1])
    percore = dict(wbd=wbd, xt=xt,
                   idxa=idx_dev[0], idxb=idx_dev[1],
                   wa=w_dev[0], wb=w_dev[1], da=d_dev[0], db=d_dev[1])
    shared = dict(wg1=wg[0], wg2=wg[1], wg3=wg[2], ws1=ws[0], ws2=ws[1], ws3=ws[2],
                  bn1=bn[0], bn2=bn[1], bn3=bn[2], iota=iota16, ones=ones_row)
    return meta, percore, shared


def _build(meta):
    from concourse import bass, bacc, mybir, tile
    from concourse.masks import make_identity

    dpad = meta["dpad"]
    C = meta["C"]
    nbat = [meta["nbat_a"], meta["nbat_b"]]
    nslot = [meta["nslot_a"], meta["nslot_b"]]
    f16, f32, i16 = mybir.dt.float16, mybir.dt.float32, mybir.dt.int16
    Alu = mybir.AluOpType
    Act = mybir.ActivationFunctionType

    nc = bacc.Bacc("TRN2", target_bir_lowering=False, debug=False,
                   num_devices=NCORES)

    # --- I/O ---
    xt = nc.dram_tensor("xt", [F_IN, NPC], f32, kind="ExternalInput")
    wbd = nc.dram_tensor("wbd", [128, NBLK, dpad], f32, kind="ExternalInput")
    idx_in = [nc.dram_tensor("idxa", [nbat[0], 32, 512], i16, kind="ExternalInput"),
              nc.dram_tensor("idxb", [nbat[1], 32, 512], i16, kind="ExternalInput")]
    w_in = [nc.dram_tensor("wa", [128, nslot[0]], f32, kind="ExternalInput"),
            nc.dram_tensor("wb", [128, nslot[1]], f32, kind="ExternalInput")]
    d_in = [nc.dram_tensor("da", [128, nslot[0]], f32, kind="ExternalInput"),
            nc.dram_tensor("db", [128, nslot[1]], f32, kind="ExternalInput")]
    wg_in = [nc.dram_tensor(f"wg{i}", [F_IN if i == 1 else UNITS, FW], f32,
                            kind="ExternalInput") for i in (1, 2, 3)]
    ws_in = [nc.dram_tensor(f"ws{i}", [F_IN if i == 1 else UNITS, FW], f32,
                            kind="ExternalInput") for i in (1, 2, 3)]
    bn_in = [nc.dram_tensor(f"bn{i}", [1, 5 * FW], f32, kind="ExternalInput")
             for i in (1, 2, 3)]
    iota_in = nc.dram_tensor("iota", [128, 128], f16, kind="ExternalInput")
    ones_in = nc.dram_tensor("ones", [1, 128], f32, kind="ExternalInput")
    out_ext = nc.dram_tensor("out", [NPC, NCLS], f32, kind="ExternalOutput")

    with tile.TileContext(nc) as tc:
        with (
            tc.tile_pool(name="pers", bufs=1) as pers,
            tc.tile_pool(name="stream", bufs=4) as stream,
            tc.tile_pool(name="spool", bufs=6) as spool,
            tc.tile_pool(name="gat", bufs=2) as gatp,
            tc.tile_pool(name="idxp", bufs=3) as idxp,
            tc.tile_pool(name="pa", bufs=2, space="PSUM") as pa,
            tc.tile_pool(name="pb", bufs=2, space="PSUM") as pb,
            tc.tile_pool(name="pt", bufs=1, space="PSUM") as pt,
            tc.tile_pool(name="ps2", bufs=1, space="PSUM") as ps2,
            tc.tile_pool(name="ptr", bufs=2, space="PSUM") as ptr,
            tc.tile_pool(name="dram", bufs=1, space="DRAM") as dram,
        ):
            # ---------- setup ----------
            iota_t = pers.tile([128, 128], f16, tag="iota", name="iota")
            nc.sync.dma_start(iota_t[:], iota_in[:])
            ones_t = pers.tile([1, 128], f32, tag="ones", name="ones")
            nc.sync.dma_start(ones_t[:], ones_in[:])
            ident = pers.tile([128, 128], f32, tag="ident", name="ident")
            make_identity(nc, ident[:])

            wt = [pers.tile([128, nslot[h]], f32, tag=f"w{h}", name=f"w{h}")
                  for h in (0, 1)]
            dt_ = [pers.tile([128, nslot[h]], f32, tag=f"d{h}", name=f"d{h}")
                   for h in (0, 1)]
            for h in (0, 1):
                nc.sync.dma_start(wt[h][:], w_in[h][:])
                nc.sync.dma_start(dt_[h][:], d_in[h][:])

            # degree -> dinv / dinvinv  [128, NBLK]
            wbd_t = pers.tile([128, NBLK, dpad], f32, tag="wbd", name="wbd")
            nc.sync.dma_start(wbd_t[:], wbd[:])
            epsc = pers.tile([128, 1], f32, tag="epsc", name="epsc")
            nc.vector.memset(epsc[:], 1e-30)
            epsr = pers.tile([1, 1], f32, tag="epsr", name="epsr")
            nc.vector.memset(epsr[:], EPS)
            deg = pers.tile([128, NBLK], f32, tag="deg", name="deg")
            nc.vector.tensor_reduce(deg[:], wbd_t[:], axis=mybir.AxisListType.X,
                                    op=Alu.add)
            sq = pers.tile([128, NBLK], f32, tag="sq", name="sq")
            nc.scalar.activation(sq[:], deg[:], Act.Sqrt, bias=epsc[:, 0:1])
            rec = pers.tile([128, NBLK], f32, tag="rec", name="rec")
            nc.vector.reciprocal(rec[:], sq[:])
            mask = pers.tile([128, NBLK], f32, tag="mask", name="mask")
            nc.vector.tensor_scalar(mask[:], deg[:], 0.0, None, Alu.is_gt)
            dinv = pers.tile([128, NBLK], f32, tag="dinv", name="dinv")
            nc.vector.tensor_tensor(out=dinv[:], in0=rec[:], in1=mask[:], op=Alu.mult)
            dinvinv = pers.tile([128, NBLK], f32, tag="dinvinv", name="dinvinv")
            nc.vector.tensor_tensor(out=dinvinv[:], in0=sq[:], in1=mask[:], op=Alu.mult)

            # BN folding per layer
            grep, c0row, wgp, wsp = [], [], [], []
            for li in range(3):
                fi = F_IN if li == 0 else UNITS
                bnt = pers.tile([1, 5 * FW], f32, tag=f"bn{li}", name=f"bn{li}")
                nc.sync.dma_start(bnt[:], bn_in[li][:])
                sqv = pers.tile([1, FW], f32, tag=f"sqv{li}", name=f"sqv{li}")
                nc.scalar.activation(sqv[:], bnt[:, 4 * FW : 5 * FW], Act.Sqrt,
                                     bias=epsr[:, 0:1])
                recv = pers.tile([1, FW], f32, tag=f"recv{li}", name=f"recv{li}")
                nc.vector.reciprocal(recv[:], sqv[:])
                gr = pers.tile([1, FW], f32, tag=f"grow{li}", name=f"grow{li}")
                nc.vector.tensor_tensor(out=gr[:], in0=recv[:],
                                        in1=bnt[:, FW : 2 * FW], op=Alu.mult)
                c0 = pers.tile([1, FW], f32, tag=f"c0{li}", name=f"c0{li}")
                t1 = pers.tile([1, FW], f32, tag=f"t1{li}", name=f"t1{li}")
                nc.vector.tensor_tensor(out=t1[:], in0=bnt[:, 0:FW],
                                        in1=bnt[:, 3 * FW : 4 * FW], op=Alu.subtract)
                nc.vector.tensor_tensor(out=t1[:], in0=t1[:], in1=gr[:], op=Alu.mult)
                nc.vector.tensor_tensor(out=c0[:], in0=t1[:],
                                        in1=bnt[:, 2 * FW : 3 * FW], op=Alu.add)
                c0row.append(c0)
                gp_ps = pt.tile([128, FW], f32, tag="ps_t", name="gp_ps")
                nc.tensor.matmul(gp_ps[:], ones_t[:], gr[:], start=True, stop=True)
                gp = pers.tile([128, FW], f32, tag=f"grep{li}", name=f"grep{li}")
                nc.vector.tensor_copy(out=gp[:], in_=gp_ps[:])
                grep.append(gp)

                ntile = fi // 128
                wgl, wsl = [], []
                for k in range(ntile):
                    for (dst_list, src_dram, nm) in ((wgl, wg_in[li], "wg"),
                                                    (wsl, ws_in[li], "ws")):
                        raw = stream.tile([128, FW], f32, tag="wraw", name="wraw")
                        nc.sync.dma_start(raw[:], src_dram[k * 128 : (k + 1) * 128, :])
                        wp = pers.tile([128, FW], f32, tag=f"{nm}p{li}_{k}", name=f"{nm}p{li}_{k}")
                        nc.vector.tensor_tensor(out=wp[:], in0=raw[:], in1=gp[:],
                                                op=Alu.mult)
                        dst_list.append(wp)
                wgp.append(wgl)
                wsp.append(wsl)

            # persistent big buffers
            accA = pers.tile([128, NBLK * FW], f32, tag="accA", name="accA")
            selfb = pers.tile([128, NBLK * FW], f32, tag="selfb", name="selfb")
            tbls = pers.tile([128, NBLK, FW], f16, tag="tbls", name="tbls")
            hT = [pers.tile([128, NPC], f32, tag="hT1", name="hT1"),
                  pers.tile([128, NPC], f32, tag="hT2", name="hT2")]

            # ---------- layers ----------
            for li in range(3):
                fi = F_IN if li == 0 else UNITS
                ktiles = fi // 128
                relu = li < 2

                # table + self build per block
                for j in range(NBLK):
                    if li == 0:
                        lhs = []
                        for k in range(ktiles):
                            xa = stream.tile([128, 128], f32, tag="xs", name="xs")
                            nc.sync.dma_start(
                                xa[:], xt[k * 128 : (k + 1) * 128,
                                          j * 128 : (j + 1) * 128])
                            lhs.append(xa)
                    else:
                        lhs = [hT[li - 1][:, j * 128 : (j + 1) * 128]]
                    ps_t = pt.tile([128, FW], f32, tag="ps_t", name="ps_t")
                    ps_s = ps2.tile([128, FW], f32, tag="ps_s", name="ps_s")
                    for k in range(ktiles):
                        lk = lhs[k][:] if li == 0 else lhs[k]
                        nc.tensor.matmul(ps_t[:], lk, wgp[li][k][:],
                                         start=(k == 0), stop=(k == ktiles - 1))
                        nc.tensor.matmul(ps_s[:], lk, wsp[li][k][:],
                                         start=(k == 0), stop=False)
                    nc.tensor.matmul(ps_s[:], ones_t[:], c0row[li][:],
                                     start=False, stop=True)
                    nc.scalar.activation(tbls[:, j, :], ps_t[:], Act.Copy,
                                         scale=dinv[:, j : j + 1])
                    nc.scalar.activation(selfb[:, j * FW : (j + 1) * FW], ps_s[:],
                                         Act.Copy, scale=dinvinv[:, j : j + 1])

                # stage table halves to DRAM, AllGather each half
                t_in = [dram.tile([HALF, FW], f16, tag=f"tin{li}{h}", name=f"tin{li}{h}") for h in (0, 1)]
                t_out = [dram.tile([NCORES * HALF, FW], f16, tag=f"tout{li}{h}", name=f"tout{li}{h}")
                         for h in (0, 1)]
                nc.sync.dma_start(
                    t_in[0][0:3072].rearrange("(j p) f -> p j f", p=128),
                    tbls[:, 0:24, :])
                nc.sync.dma_start(t_in[0][3072:3136], tbls[0:64, 24, :])
                nc.sync.dma_start(t_in[1][0:64], tbls[64:128, 24, :])
                nc.sync.dma_start(
                    t_in[1][64:3136].rearrange("(j p) f -> p j f", p=128),
                    tbls[:, 25:49, :])
                for h in (0, 1):
                    nc.gpsimd.collective_compute(
                        "AllGather", Alu.bypass,
                        replica_groups=[list(range(NCORES))],
                        ins=[t_in[h].opt()], outs=[t_out[h].opt()])

                # phases
                for h in (0, 1):
                    gts = []
                    for b in range(nbat[h]):
                        it = idxp.tile([128, 512], i16, tag="idx", name="idx")
                        nc.sync.dma_start(it[0:32, :], idx_in[h][b, :, :])
                        gt = gatp.tile([128, BATCH, FW], f16, tag="gat", name="gat")
                        nc.gpsimd.dma_gather(gt[:], t_out[h][:], it[:],
                                             BATCH * 128, BATCH * 128, FW,
                                             single_packet=False)
                        gts.append(gt)
                    c = 0
                    for j in range(NBLK):
                        nchunks = int(C[j, h])
                        if h == 0:
                            psm = pa.tile([128, FW], f32, tag="psa", name="psa")
                        else:
                            psm = pb.tile([128, FW], f32, tag="psb", name="psb")
                            nc.tensor.matmul(psm[:], ident[:],
                                             accA[:, j * FW : (j + 1) * FW],
                                             start=True, stop=False)
                            nc.tensor.matmul(psm[:], ident[:],
                                             selfb[:, j * FW : (j + 1) * FW],
                                             start=False, stop=False)
                        for k in range(nchunks):
                            S = spool.tile([128, 128], f16, tag="S", name="S")
                            nc.vector.tensor_scalar(
                                S[:], iota_t[:],
                                dt_[h][:, c : c + 1], wt[h][:, c : c + 1],
                                Alu.is_equal, Alu.mult)
                            nc.tensor.matmul(
                                psm[:], S[:], gts[c // BATCH][:, c % BATCH, :],
                                start=(h == 0 and k == 0),
                                stop=(k == nchunks - 1))
                            c += 1
                        if h == 0:
                            nc.scalar.activation(accA[:, j * FW : (j + 1) * FW],
                                                 psm[:], Act.Copy)
                        else:
                            hblk = stream.tile([128, FW], f32, tag="hblk", name="hblk")
                            nc.scalar.activation(
                                hblk[:], psm[:],
                                Act.Relu if relu else Act.Copy,
                                scale=dinv[:, j : j + 1])
                            if li < 2:
                                tr = ptr.tile([128, 128], f32, tag="tr", name="tr")
                                nc.tensor.transpose(tr[:], hblk[:], ident[:])
                                nc.scalar.copy(hT[li][:, j * 128 : (j + 1) * 128],
                                               tr[:])
                            else:
                                nc.sync.dma_start(
                                    out_ext[j * 128 : (j + 1) * 128, :],
                                    hblk[:, 0:NCLS])

    nc.compile()
    return nc


_CACHE = {}


def _install_ntff_shim():
    try:
        import antenv
        if "antenv.axon_hooks" in sys.modules:
            return
        mod = types.ModuleType("antenv.axon_hooks")
        hook = [None]
        mod.set_axon_ntff_profile_hook = lambda h: hook.__setitem__(0, h)
        mod.get_axon_ntff_profile_hook = lambda: hook[0]
        sys.modules["antenv.axon_hooks"] = mod
        antenv.axon_hooks = mod
        from trn_agent_boot.trn_boot import _ntff_profile_via_ctypes
        mod.set_axon_ntff_profile_hook(
            _ntff_profile_via_ctypes("/opt/axon/libaxon_pjrt.so"))
    except Exception:
        pass


def kernel(trace=False, **inputs):
    from concourse import bass_utils

    meta, percore, shared = _host_prep(inputs)
    key = (meta["dpad"], meta["nbat_a"], meta["nbat_b"], meta["C"].tobytes())
    if key not in _CACHE:
        _CACHE[key] = _build(meta)
    nc = _CACHE[key]

    in_maps = []
    for c in range(NCORES):
        m = dict(shared)
        m["xt"] = percore["xt"][c]
        m["wbd"] = percore["wbd"][c]
        m["idxa"] = percore["idxa"][c]
        m["idxb"] = percore["idxb"][c]
        m["wa"] = percore["wa"][c]
        m["wb"] = percore["wb"][c]
        m["da"] = percore["da"][c]
        m["db"] = percore["db"][c]
        in_maps.append(m)

    if trace:
        _install_ntff_shim()
    res = bass_utils.run_bass_kernel_spmd(
        nc, in_maps, core_ids=list(range(NCORES)), trace=trace)
    full = np.concatenate([res.results[c]["out"] for c in range(NCORES)], axis=0)
    out = full[:N].astype(np.float32)
    if trace:
        kernel.last_exec_time_ns = res.exec_time_ns
    return out


# revision 9
# speedup vs baseline: 1.5519x; 1.5519x over previous
"""DropEdge GCN (3-layer, inference) on 8 Trainium2 NeuronCores.

Strategy: partition nodes across the 8 cores by destination (graph parallel).
Per layer, each core builds its slice of the fp16 "message table"
T = dinv * (h @ (W_gcn * g_bn)) (BN scale folded into the weights), two
AllGathers replicate T into every core's DRAM, then dma_gather pulls the
source rows for 128-edge chunks, a one-hot*weight matrix S (built on the
vector engine from iota==dst_local) turns the segment-sum into PSUM-
accumulating matmuls S^T @ M per 128-node destination block. The self-loop
term, the folded BN bias, and the per-node dinv scaling ride the table-build
matmuls / the PSUM-evicting activation op.
"""
import sys
import types
import numpy as np

N = 50000
E = 1_000_000
F_IN = 256
UNITS = 128
NCLS = 40
EPS = 1e-3

NCORES = 8
NPC = 6272            # padded nodes per core
NBT = NCORES * NPC    # 50176
NBLK = 49             # 128-node blocks per core
HALF = 3136           # phase split within a core's node range
FW = 128              # uniform table width (layer-3 output padded 40->128)
BATCH = 8             # chunks per dma_gather call (8*128 = 1024 rows)


def _pad_cols(a, width):
    out = np.zeros((a.shape[0], width), np.float32)
    out[:, : a.shape[1]] = a
    return out


def _host_prep(inputs):
    x = np.asarray(inputs["x"], np.float32)
    ei = np.asarray(inputs["edge_index"])
    w = np.asarray(inputs["edge_weight"], np.float32)
    src = ei[0].astype(np.int64)
    dst = ei[1].astype(np.int64)

    counts = np.bincount(dst, minlength=NBT)
    assert counts[:N].min() > 0, "degree-0 real node: unsupported fast path"
    dpad = int(-(-int(counts.max()) // 4) * 4)

    # weights grouped by dst node (padded CSR) for the on-device degree reduce
    order0 = np.argsort(dst, kind="stable")
    ofs = np.zeros(NBT + 1, np.int64)
    ofs[1:] = np.cumsum(counts)
    slot = np.arange(E) - ofs[dst[order0]]
    wbd_full = np.zeros((NBT, dpad), np.float32)
    wbd_full[dst[order0], slot] = w[order0]
    wbd = np.ascontiguousarray(
        wbd_full.reshape(NCORES, NBLK, 128, dpad).transpose(0, 2, 1, 3)
    )  # [NC, 128, NBLK, dpad]

    # edge groups by (dst core, dst block, src half)
    core_e = dst // NPC
    blk_e = (dst % NPC) // 128
    dloc = (dst % 128).astype(np.float32)
    hsrc = (src % NPC) // HALF
    srow = ((src // NPC) * HALF + (src % NPC) % HALF).astype(np.int64)
    assert srow.max() < NCORES * HALF

    key = (core_e * NBLK + blk_e) * 2 + hsrc
    order = np.argsort(key, kind="stable")
    gcnt = np.bincount(key, minlength=NCORES * NBLK * 2).reshape(NCORES, NBLK, 2)
    gofs = np.zeros(NCORES * NBLK * 2 + 1, np.int64)
    gofs[1:] = np.cumsum(gcnt.reshape(-1))
    C = np.maximum(1, -(-gcnt // 128)).max(axis=0)  # [NBLK, 2] shared chunk counts

    nch = [int(C[:, h].sum()) for h in (0, 1)]
    nbat = [-(-nch[h] // BATCH) for h in (0, 1)]
    nslot = [nbat[h] * BATCH for h in (0, 1)]

    idx_arr = [np.zeros((NCORES, nslot[h] * 128), np.int64) for h in (0, 1)]
    w_arr = [np.zeros((NCORES, nslot[h] * 128), np.float32) for h in (0, 1)]
    d_arr = [np.zeros((NCORES, nslot[h] * 128), np.float32) for h in (0, 1)]
    for c in range(NCORES):
        for h in (0, 1):
            pos = 0
            for j in range(NBLK):
                g = (c * NBLK + j) * 2 + h
                k = gcnt[c, j, h]
                es = order[gofs[g] : gofs[g] + k]
                base = pos * 128
                idx_arr[h][c, base : base + k] = srow[es]
                w_arr[h][c, base : base + k] = w[es]
                d_arr[h][c, base : base + k] = dloc[es]
                pos += int(C[j, h])

    # device layouts
    idx_dev, w_dev, d_dev = [], [], []
    for h in (0, 1):
        ia = np.zeros((NCORES, nbat[h], 32, BATCH * 8), np.int16)
        for c in range(NCORES):
            fl = idx_arr[h][c].astype(np.int16).reshape(nbat[h], BATCH * 128)
            wrapped = fl.reshape(nbat[h], BATCH * 8, 16).transpose(0, 2, 1)
            ia[c, :, 0:16] = wrapped
            ia[c, :, 16:32] = wrapped
        idx_dev.append(ia)
        w_dev.append(
            np.ascontiguousarray(
                w_arr[h].reshape(NCORES, nslot[h], 128).transpose(0, 2, 1)
            )
        )
        d_dev.append(
            np.ascontiguousarray(
                d_arr[h].reshape(NCORES, nslot[h], 128).transpose(0, 2, 1)
            )
        )

    # node features, transposed + padded, per-core column slice
    xt_full = np.zeros((F_IN, NBT), np.float32)
    xt_full[:, :N] = x.T
    xt = np.ascontiguousarray(xt_full.reshape(F_IN, NCORES, NPC).transpose(1, 0, 2))

    # weights (pad layer 3 to 128 wide)
    wg = [
        np.asarray(inputs["w_gcn1"], np.float32),
        np.asarray(inputs["w_gcn2"], np.float32),
        _pad_cols(np.asarray(inputs["w_gcn3"], np.float32), FW),
    ]
    ws = [
        np.asarray(inputs["w_self1"], np.float32),
        np.asarray(inputs["w_self2"], np.float32),
        _pad_cols(np.asarray(inputs["w_self3"], np.float32), FW),
    ]
    bn = []
    for li in (1, 2, 3):
        row = np.zeros((1, 5 * FW), np.float32)
        fo = UNITS if li < 3 else NCLS
        row[0, 0 * FW : 0 * FW + fo] = np.asarray(inputs[f"b{li}"], np.float32)
        row[0, 1 * FW : 1 * FW + fo] = np.asarray(inputs[f"gamma{li}"], np.float32)
        row[0, 2 * FW : 2 * FW + fo] = np.asarray(inputs[f"beta{li}"], np.float32)
        row[0, 3 * FW : 3 * FW + fo] = np.asarray(inputs[f"mean{li}"], np.float32)
        row[0, 4 * FW : 4 * FW + fo] = 1.0
        row[0, 4 * FW : 4 * FW + fo] = np.asarray(inputs[f"var{li}"], np.float32)
        row[0, 4 * FW + fo : 5 * FW] = 1.0  # padded var=1 avoids rsqrt(eps) blowup
        bn.append(row)

    iota16 = np.tile(np.arange(128, dtype=np.float16)[None, :], (128, 1))
    ones_row = np.ones((1, 128), np.float32)

    meta = dict(dpad=dpad, C=C, nbat_a=nbat[0], nbat_b=nbat[1],
                nslot_a=nslot[0], nslot_b=nslot[1])
    percore = dict(wbd=wbd, xt=xt,
                   idxa=idx_dev[0], idxb=idx_dev[1],
                   wa=w_dev[0], wb=w_dev[1], da=d_dev[0], db=d_dev[1])
    shared = dict(wg1=wg[0], wg2=wg[1], wg3=wg[2], ws1=ws[0], ws2=ws[1], ws3=ws[2],
                  bn1=bn[0], bn2=bn[1], bn3=bn[2], iota=iota16, ones=ones_row)
    return meta, percore, shared


def _build(meta):
    from concourse import bass, bacc, mybir, tile
    from concourse.masks import make_identity

    dpad = meta["dpad"]
    C = meta["C"]
    nbat = [meta["nbat_a"], meta["nbat_b"]]
    nslot = [meta["nslot_a"], meta["nslot_b"]]
    f16, f32, i16 = mybir.dt.float16, mybir.dt.float32, mybir.dt.int16
    Alu = mybir.AluOpType
    Act = mybir.ActivationFunctionType

    nc = bacc.Bacc("TRN2", target_bir_lowering=False, debug=False,
                   num_devices=NCORES)

    # --- I/O ---
    xt = nc.dram_tensor("xt", [F_IN, NPC], f32, kind="ExternalInput")
    wbd = nc.dram_tensor("wbd", [128, NBLK, dpad], f32, kind="ExternalInput")
    idx_in = [nc.dram_tensor("idxa", [nbat[0], 32, BATCH * 8], i16, kind="ExternalInput"),
              nc.dram_tensor("idxb", [nbat[1], 32, BATCH * 8], i16, kind="ExternalInput")]
    w_in = [nc.dram_tensor("wa", [128, nslot[0]], f32, kind="ExternalInput"),
            nc.dram_tensor("wb", [128, nslot[1]], f32, kind="ExternalInput")]
    d_in = [nc.dram_tensor("da", [128, nslot[0]], f32, kind="ExternalInput"),
            nc.dram_tensor("db", [128, nslot[1]], f32, kind="ExternalInput")]
    wg_in = [nc.dram_tensor(f"wg{i}", [F_IN if i == 1 else UNITS, FW], f32,
                            kind="ExternalInput") for i in (1, 2, 3)]
    ws_in = [nc.dram_tensor(f"ws{i}", [F_IN if i == 1 else UNITS, FW], f32,
                            kind="ExternalInput") for i in (1, 2, 3)]
    bn_in = [nc.dram_tensor(f"bn{i}", [1, 5 * FW], f32, kind="ExternalInput")
             for i in (1, 2, 3)]
    iota_in = nc.dram_tensor("iota", [128, 128], f16, kind="ExternalInput")
    ones_in = nc.dram_tensor("ones", [1, 128], f32, kind="ExternalInput")
    out_ext = nc.dram_tensor("out", [NPC, NCLS], f32, kind="ExternalOutput")

    with tile.TileContext(nc) as tc:
        with (
            tc.tile_pool(name="pers", bufs=1) as pers,
            tc.tile_pool(name="stream", bufs=4) as stream,
            tc.tile_pool(name="spool", bufs=6) as spool,
            tc.tile_pool(name="gat", bufs=8) as gatp,
            tc.tile_pool(name="idxp", bufs=8) as idxp,
            tc.tile_pool(name="pa", bufs=2, space="PSUM") as pa,
            tc.tile_pool(name="pb", bufs=2, space="PSUM") as pb,
            tc.tile_pool(name="pt", bufs=1, space="PSUM") as pt,
            tc.tile_pool(name="ps2", bufs=1, space="PSUM") as ps2,
            tc.tile_pool(name="ptr", bufs=2, space="PSUM") as ptr,
            tc.tile_pool(name="dram", bufs=1, space="DRAM") as dram,
        ):
            # ---------- setup ----------
            iota_t = pers.tile([128, 128], f16, tag="iota", name="iota")
            nc.sync.dma_start(iota_t[:], iota_in[:])
            ones_t = pers.tile([1, 128], f32, tag="ones", name="ones")
            nc.sync.dma_start(ones_t[:], ones_in[:])
            ident = pers.tile([128, 128], f32, tag="ident", name="ident")
            make_identity(nc, ident[:])

            wt = [pers.tile([128, nslot[h]], f32, tag=f"w{h}", name=f"w{h}")
                  for h in (0, 1)]
            dt_ = [pers.tile([128, nslot[h]], f32, tag=f"d{h}", name=f"d{h}")
                   for h in (0, 1)]
            for h in (0, 1):
                nc.sync.dma_start(wt[h][:], w_in[h][:])
                nc.sync.dma_start(dt_[h][:], d_in[h][:])

            # degree -> dinv / dinvinv  [128, NBLK]
            wbd_t = pers.tile([128, NBLK, dpad], f32, tag="wbd", name="wbd")
            nc.sync.dma_start(wbd_t[:], wbd[:])
            epsc = pers.tile([128, 1], f32, tag="epsc", name="epsc")
            nc.vector.memset(epsc[:], 1e-30)
            epsr = pers.tile([1, 1], f32, tag="epsr", name="epsr")
            nc.vector.memset(epsr[:], EPS)
            deg = pers.tile([128, NBLK], f32, tag="deg", name="deg")
            nc.vector.tensor_reduce(deg[:], wbd_t[:], axis=mybir.AxisListType.X,
                                    op=Alu.add)
            sq = pers.tile([128, NBLK], f32, tag="sq", name="sq")
            nc.scalar.activation(sq[:], deg[:], Act.Sqrt, bias=epsc[:, 0:1])
            rec = pers.tile([128, NBLK], f32, tag="rec", name="rec")
            nc.vector.reciprocal(rec[:], sq[:])
            mask = pers.tile([128, NBLK], f32, tag="mask", name="mask")
            nc.vector.tensor_scalar(mask[:], deg[:], 0.0, None, Alu.is_gt)
            dinv = pers.tile([128, NBLK], f32, tag="dinv", name="dinv")
            nc.vector.tensor_tensor(out=dinv[:], in0=rec[:], in1=mask[:], op=Alu.mult)
            dinvinv = pers.tile([128, NBLK], f32, tag="dinvinv", name="dinvinv")
            nc.vector.tensor_tensor(out=dinvinv[:], in0=sq[:], in1=mask[:], op=Alu.mult)

            # BN folding per layer
            grep, c0row, wgp, wsp = [], [], [], []
            for li in range(3):
                fi = F_IN if li == 0 else UNITS
                bnt = pers.tile([1, 5 * FW], f32, tag=f"bn{li}", name=f"bn{li}")
                nc.sync.dma_start(bnt[:], bn_in[li][:])
                sqv = pers.tile([1, FW], f32, tag=f"sqv{li}", name=f"sqv{li}")
                nc.scalar.activation(sqv[:], bnt[:, 4 * FW : 5 * FW], Act.Sqrt,
                                     bias=epsr[:, 0:1])
                recv = pers.tile([1, FW], f32, tag=f"recv{li}", name=f"recv{li}")
                nc.vector.reciprocal(recv[:], sqv[:])
                gr = pers.tile([1, FW], f32, tag=f"grow{li}", name=f"grow{li}")
                nc.vector.tensor_tensor(out=gr[:], in0=recv[:],
                                        in1=bnt[:, FW : 2 * FW], op=Alu.mult)
                c0 = pers.tile([1, FW], f32, tag=f"c0{li}", name=f"c0{li}")
                t1 = pers.tile([1, FW], f32, tag=f"t1{li}", name=f"t1{li}")
                nc.vector.tensor_tensor(out=t1[:], in0=bnt[:, 0:FW],
                                        in1=bnt[:, 3 * FW : 4 * FW], op=Alu.subtract)
                nc.vector.tensor_tensor(out=t1[:], in0=t1[:], in1=gr[:], op=Alu.mult)
                nc.vector.tensor_tensor(out=c0[:], in0=t1[:],
                                        in1=bnt[:, 2 * FW : 3 * FW], op=Alu.add)
                c0row.append(c0)
                gp_ps = pt.tile([128, FW], f32, tag="ps_t", name="gp_ps")
                nc.tensor.matmul(gp_ps[:], ones_t[:], gr[:], start=True, stop=True)
                gp = pers.tile([128, FW], f32, tag=f"grep{li}", name=f"grep{li}")
                nc.vector.tensor_copy(out=gp[:], in_=gp_ps[:])
                grep.append(gp)

                ntile = fi // 128
                wgl, wsl = [], []
                for k in range(ntile):
                    for (dst_list, src_dram, nm) in ((wgl, wg_in[li], "wg"),
                                                    (wsl, ws_in[li], "ws")):
                        raw = stream.tile([128, FW], f32, tag="wraw", name="wraw")
                        nc.sync.dma_start(raw[:], src_dram[k * 128 : (k + 1) * 128, :])
                        wp = pers.tile([128, FW], f32, tag=f"{nm}p{li}_{k}", name=f"{nm}p{li}_{k}")
                        nc.vector.tensor_tensor(out=wp[:], in0=raw[:], in1=gp[:],
                                                op=Alu.mult)
                        dst_list.append(wp)
                wgp.append(wgl)
                wsp.append(wsl)

            # persistent big buffers
            accA = pers.tile([128, NBLK * FW], f32, tag="accA", name="accA")
            selfb = pers.tile([128, NBLK * FW], f32, tag="selfb", name="selfb")
            tbls = pers.tile([128, NBLK, FW], f16, tag="tbls", name="tbls")
            hT = [pers.tile([128, NPC], f32, tag="hT1", name="hT1"),
                  pers.tile([128, NPC], f32, tag="hT2", name="hT2")]

            # ---------- layers ----------
            t_in = {}
            t_out = {}

            def emit_table_slice(li, j, lhs):
                # lhs: list of [128, 128] APs (K-tiles of h^T or x^T for block j)
                ktiles = len(lhs)
                ps_t = pt.tile([128, FW], f32, tag="ps_t", name="ps_t")
                ps_s = ps2.tile([128, FW], f32, tag="ps_s", name="ps_s")
                for k in range(ktiles):
                    nc.tensor.matmul(ps_t[:], lhs[k], wgp[li][k][:],
                                     start=(k == 0), stop=(k == ktiles - 1))
                    nc.tensor.matmul(ps_s[:], lhs[k], wsp[li][k][:],
                                     start=(k == 0), stop=False)
                nc.tensor.matmul(ps_s[:], ones_t[:], c0row[li][:],
                                 start=False, stop=True)
                nc.scalar.activation(tbls[:, j, :], ps_t[:], Act.Copy,
                                     scale=dinv[:, j : j + 1])
                nc.scalar.activation(selfb[:, j * FW : (j + 1) * FW], ps_s[:],
                                     Act.Copy, scale=dinvinv[:, j : j + 1])

            def emit_ag(li, h):
                # stage the finished table half to DRAM and AllGather it
                ti = dram.tile([HALF, FW], f16, tag=f"tin{li}{h}",
                               name=f"tin{li}{h}")
                to = dram.tile([NCORES * HALF, FW], f16, tag=f"tout{li}{h}",
                               name=f"tout{li}{h}")
                t_in[(li, h)] = ti
                t_out[(li, h)] = to
                if h == 0:
                    nc.sync.dma_start(
                        ti[0:3072].rearrange("(j p) f -> p j f", p=128),
                        tbls[:, 0:24, :])
                    nc.sync.dma_start(ti[3072:3136], tbls[0:64, 24, :])
                else:
                    nc.sync.dma_start(ti[0:64], tbls[64:128, 24, :])
                    nc.sync.dma_start(
                        ti[64:3136].rearrange("(j p) f -> p j f", p=128),
                        tbls[:, 25:49, :])
                nc.gpsimd.collective_compute(
                    "AllGather", Alu.bypass,
                    replica_groups=[list(range(NCORES))],
                    ins=[ti.opt()], outs=[to.opt()])

            # layer-1 table from x^T (streamed from DRAM)
            for j in range(NBLK):
                lhs = []
                for k in range(2):
                    xa = stream.tile([128, 128], f32, tag="xs", name="xs")
                    nc.sync.dma_start(
                        xa[:], xt[k * 128 : (k + 1) * 128,
                                  j * 128 : (j + 1) * 128])
                    lhs.append(xa[:])
                emit_table_slice(0, j, lhs)
                if j == 24:
                    emit_ag(0, 0)
                if j == NBLK - 1:
                    emit_ag(0, 1)

            for li in range(3):
                relu = li < 2
                for h in (0, 1):
                    gts = []
                    for b in range(nbat[h]):
                        it = idxp.tile([128, BATCH * 8], i16, tag="idx", name="idx")
                        nc.sync.dma_start(it[0:32, :], idx_in[h][b, :, :])
                        gt = gatp.tile([128, BATCH, FW], f16, tag="gat", name="gat")
                        nc.gpsimd.dma_gather(gt[:], t_out[(li, h)][:], it[:],
                                             BATCH * 128, BATCH * 128, FW,
                                             single_packet=True)
                        gts.append(gt)
                    c = 0
                    for j in range(NBLK):
                        nchunks = int(C[j, h])
                        if h == 0:
                            psm = pa.tile([128, FW], f32, tag="psa", name="psa")
                        else:
                            psm = pb.tile([128, FW], f32, tag="psb", name="psb")
                            nc.tensor.matmul(psm[:], ident[:],
                                             accA[:, j * FW : (j + 1) * FW],
                                             start=True, stop=False)
                            nc.tensor.matmul(psm[:], ident[:],
                                             selfb[:, j * FW : (j + 1) * FW],
                                             start=False, stop=False)
                        for k in range(nchunks):
                            S = spool.tile([128, 128], f16, tag="S", name="S")
                            nc.vector.tensor_scalar(
                                S[:], iota_t[:],
                                dt_[h][:, c : c + 1], wt[h][:, c : c + 1],
                                Alu.is_equal, Alu.mult)
                            nc.tensor.matmul(
                                psm[:], S[:], gts[c // BATCH][:, c % BATCH, :],
                                start=(h == 0 and k == 0),
                                stop=(k == nchunks - 1))
                            c += 1
                        if h == 0:
                            nc.scalar.activation(accA[:, j * FW : (j + 1) * FW],
                                                 psm[:], Act.Copy)
                        else:
                            hblk = stream.tile([128, FW], f32, tag="hblk",
                                               name="hblk")
                            nc.scalar.activation(
                                hblk[:], psm[:],
                                Act.Relu if relu else Act.Copy,
                                scale=dinv[:, j : j + 1])
                            if li < 2:
                                tr = ptr.tile([128, 128], f32, tag="tr", name="tr")
                                nc.tensor.transpose(tr[:], hblk[:], ident[:])
                                nc.scalar.copy(hT[li][:, j * 128 : (j + 1) * 128],
                                               tr[:])
                                # next layer's table slice rides phase B
                                emit_table_slice(
                                    li + 1, j,
                                    [hT[li][:, j * 128 : (j + 1) * 128]])
                                if j == 24:
                                    emit_ag(li + 1, 0)
                                if j == NBLK - 1:
                                    emit_ag(li + 1, 1)
                            else:
                                nc.sync.dma_start(
                                    out_ext[j * 128 : (j + 1) * 128, :],
                                    hblk[:, 0:NCLS])

    nc.compile()
    return nc


_CACHE = {}


def _install_ntff_shim():
    try:
        import antenv
        if "antenv.axon_hooks" in sys.modules:
            return
        mod = types.ModuleType("antenv.axon_hooks")
        hook = [None]
        mod.set_axon_ntff_profile_hook = lambda h: hook.__setitem__(0, h)
        mod.get_axon_ntff_profile_hook = lambda: hook[0]
        sys.modules["antenv.axon_hooks"] = mod
        antenv.axon_hooks = mod
        from trn_agent_boot.trn_boot import _ntff_profile_via_ctypes
        mod.set_axon_ntff_profile_hook(
            _ntff_profile_via_ctypes("/opt/axon/libaxon_pjrt.so"))
    except Exception:
        pass


def kernel(trace=False, **inputs):
    from concourse import bass_utils

    meta, percore, shared = _host_prep(inputs)
    key = (meta["dpad"], meta["nbat_a"], meta["nbat_b"], meta["C"].tobytes())
    if key not in _CACHE:
        _CACHE[key] = _build(meta)
    nc = _CACHE[key]

    in_maps = []
    for c in range(NCORES):
        m = dict(shared)
        m["xt"] = percore["xt"][c]
        m["wbd"] = percore["wbd"][c]
        m["idxa"] = percore["idxa"][c]
        m["idxb"] = percore["idxb"][c]
        m["wa"] = percore["wa"][c]
        m["wb"] = percore["wb"][c]
        m["da"] = percore["da"][c]
        m["db"] = percore["db"][c]
        in_maps.append(m)

    if trace:
        _install_ntff_shim()
    res = bass_utils.run_bass_kernel_spmd(
        nc, in_maps, core_ids=list(range(NCORES)), trace=trace)
    full = np.concatenate([res.results[c]["out"] for c in range(NCORES)], axis=0)
    out = full[:N].astype(np.float32)
    if trace:
        kernel.last_exec_time_ns = res.exec_time_ns
    return out


# revision 15
# speedup vs baseline: 1.6114x; 1.0383x over previous
"""DropEdge GCN (3-layer, inference) on 8 Trainium2 NeuronCores.

Strategy: partition nodes across the 8 cores by destination (graph parallel).
Per layer, each core builds its slice of the fp16 "message table"
T = dinv * (h @ (W_gcn * g_bn)) (BN scale folded into the weights), two
AllGathers replicate T into every core's DRAM, then dma_gather pulls the
source rows for 128-edge chunks, a one-hot*weight matrix S (built on the
vector engine from iota==dst_local) turns the segment-sum into PSUM-
accumulating matmuls S^T @ M per 128-node destination block. The self-loop
term, the folded BN bias, and the per-node dinv scaling ride the table-build
matmuls / the PSUM-evicting activation op.
"""
import sys
import types
import numpy as np

N = 50000
E = 1_000_000
F_IN = 256
UNITS = 128
NCLS = 40
EPS = 1e-3

NCORES = 8
NPC = 6272            # padded nodes per core
NBT = NCORES * NPC    # 50176
NBLK = 49             # 128-node blocks per core
HA = 4096             # phase-A table: per-core rows [0, HA)
HB = 2176             # phase-B table: per-core rows [HB, NPC)
FW = 128              # uniform table width (layer-3 output padded 40->128)
BATCH = 8             # chunks per dma_gather call (8*128 = 1024 rows)


def _pad_cols(a, width):
    out = np.zeros((a.shape[0], width), np.float32)
    out[:, : a.shape[1]] = a
    return out


def _host_prep(inputs):
    x = np.asarray(inputs["x"], np.float32)
    ei = np.asarray(inputs["edge_index"])
    w = np.asarray(inputs["edge_weight"], np.float32)
    src = ei[0].astype(np.int64)
    dst = ei[1].astype(np.int64)

    counts = np.bincount(dst, minlength=NBT)
    assert counts[:N].min() > 0, "degree-0 real node: unsupported fast path"
    dpad = int(-(-int(counts.max()) // 4) * 4)

    # weights grouped by dst node (padded CSR) for the on-device degree reduce
    order0 = np.argsort(dst, kind="stable")
    ofs = np.zeros(NBT + 1, np.int64)
    ofs[1:] = np.cumsum(counts)
    slot = np.arange(E) - ofs[dst[order0]]
    wbd_full = np.zeros((NBT, dpad), np.float32)
    wbd_full[dst[order0], slot] = w[order0]
    wbd = np.ascontiguousarray(
        wbd_full.reshape(NCORES, NBLK, 128, dpad).transpose(0, 2, 1, 3)
    )  # [NC, 128, NBLK, dpad]

    # edge groups by (dst core, dst block, src phase)
    # phase tables overlap: A = per-core rows [0, 4096), B = [2176, 6272)
    # edges whose src falls in the overlap are assigned to whichever phase
    # brings that (core, block) count to an exact multiple of 128.
    core_e = dst // NPC
    blk_e = (dst % NPC) // 128
    dloc = (dst % 128).astype(np.float32)
    sloc = src % NPC
    cls = np.where(sloc < HB, 0, np.where(sloc < HA, 1, 2)).astype(np.int64)

    key = (core_e * NBLK + blk_e) * 3 + cls
    order = np.argsort(key, kind="stable")
    kcnt = np.bincount(key, minlength=NCORES * NBLK * 3).reshape(NCORES, NBLK, 3)
    kofs = np.zeros(NCORES * NBLK * 3 + 1, np.int64)
    kofs[1:] = np.cumsum(kcnt.reshape(-1))

    na = kcnt[:, :, 0]
    nflex = kcnt[:, :, 1]
    TA = np.maximum(1, -(-na // 128)).max(axis=0)          # [NBLK] exact A chunks
    xsp = np.minimum(nflex, TA[None, :] * 128 - na)        # flex edges sent to A
    assert (xsp >= 0).all()
    cntA = na + xsp
    cntB = kcnt[:, :, 2] + (nflex - xsp)
    CB = np.maximum(1, -(-cntB // 128)).max(axis=0)        # [NBLK]
    C = np.stack([TA, CB], axis=1)                          # [NBLK, 2]

    nch = [int(C[:, h].sum()) for h in (0, 1)]
    nbat = [-(-nch[h] // BATCH) for h in (0, 1)]
    nslot = [nbat[h] * BATCH for h in (0, 1)]

    srow_a = ((src // NPC) * HA + sloc).astype(np.int64)
    srow_b = ((src // NPC) * HA + (sloc - HB)).astype(np.int64)

    idx_arr = [np.zeros((NCORES, nslot[h] * 128), np.int64) for h in (0, 1)]
    w_arr = [np.zeros((NCORES, nslot[h] * 128), np.float32) for h in (0, 1)]
    d_arr = [np.zeros((NCORES, nslot[h] * 128), np.float32) for h in (0, 1)]
    for c in range(NCORES):
        posA = 0
        posB = 0
        for j in range(NBLK):
            g = (c * NBLK + j) * 3
            e_fa = order[kofs[g] : kofs[g + 1]]
            e_fx = order[kofs[g + 1] : kofs[g + 2]]
            e_fb = order[kofs[g + 2] : kofs[g + 3]]
            xx = int(xsp[c, j])
            eA = np.concatenate([e_fa, e_fx[:xx]])
            eB = np.concatenate([e_fx[xx:], e_fb])
            for h, es, pos, srw in ((0, eA, posA, srow_a), (1, eB, posB, srow_b)):
                k = len(es)
                base = pos * 128
                idx_arr[h][c, base : base + k] = srw[es]
                w_arr[h][c, base : base + k] = w[es]
                d_arr[h][c, base : base + k] = dloc[es]
            posA += int(C[j, 0])
            posB += int(C[j, 1])

    # device layouts
    idx_dev, w_dev, d_dev = [], [], []
    for h in (0, 1):
        ia = np.zeros((NCORES, nbat[h], 32, BATCH * 8), np.int16)
        for c in range(NCORES):
            fl = idx_arr[h][c].astype(np.int16).reshape(nbat[h], BATCH * 128)
            wrapped = fl.reshape(nbat[h], BATCH * 8, 16).transpose(0, 2, 1)
            ia[c, :, 0:16] = wrapped
            ia[c, :, 16:32] = wrapped
        idx_dev.append(ia)
        w_dev.append(
            np.ascontiguousarray(
                w_arr[h].reshape(NCORES, nslot[h], 128).transpose(0, 2, 1)
            )
        )
        d_dev.append(
            np.ascontiguousarray(
                d_arr[h].reshape(NCORES, nslot[h], 128).transpose(0, 2, 1)
            )
        )

    # node features, transposed + padded, per-core column slice
    xt_full = np.zeros((F_IN, NBT), np.float32)
    xt_full[:, :N] = x.T
    xt = np.ascontiguousarray(xt_full.reshape(F_IN, NCORES, NPC).transpose(1, 0, 2))

    # weights (pad layer 3 to 128 wide)
    wg = [
        np.asarray(inputs["w_gcn1"], np.float32),
        np.asarray(inputs["w_gcn2"], np.float32),
        _pad_cols(np.asarray(inputs["w_gcn3"], np.float32), FW),
    ]
    ws = [
        np.asarray(inputs["w_self1"], np.float32),
        np.asarray(inputs["w_self2"], np.float32),
        _pad_cols(np.asarray(inputs["w_self3"], np.float32), FW),
    ]
    bn = []
    for li in (1, 2, 3):
        row = np.zeros((1, 5 * FW), np.float32)
        fo = UNITS if li < 3 else NCLS
        row[0, 0 * FW : 0 * FW + fo] = np.asarray(inputs[f"b{li}"], np.float32)
        row[0, 1 * FW : 1 * FW + fo] = np.asarray(inputs[f"gamma{li}"], np.float32)
        row[0, 2 * FW : 2 * FW + fo] = np.asarray(inputs[f"beta{li}"], np.float32)
        row[0, 3 * FW : 3 * FW + fo] = np.asarray(inputs[f"mean{li}"], np.float32)
        row[0, 4 * FW : 4 * FW + fo] = 1.0
        row[0, 4 * FW : 4 * FW + fo] = np.asarray(inputs[f"var{li}"], np.float32)
        row[0, 4 * FW + fo : 5 * FW] = 1.0  # padded var=1 avoids rsqrt(eps) blowup
        bn.append(row)

    iota16 = np.tile(np.arange(128, dtype=np.float16)[None, :], (128, 1))
    ones_row = np.ones((1, 128), np.float32)

    meta = dict(dpad=dpad, C=C, nbat_a=nbat[0], nbat_b=nbat[1],
                nslot_a=nslot[0], nslot_b=nslot[1])
    percore = dict(wbd=wbd, xt=xt,
                   idxa=idx_dev[0], idxb=idx_dev[1],
                   wa=w_dev[0], wb=w_dev[1], da=d_dev[0], db=d_dev[1])
    shared = dict(wg1=wg[0], wg2=wg[1], wg3=wg[2], ws1=ws[0], ws2=ws[1], ws3=ws[2],
                  bn1=bn[0], bn2=bn[1], bn3=bn[2], iota=iota16, ones=ones_row)
    return meta, percore, shared


def _build(meta):
    from concourse import bass, bacc, mybir, tile
    from concourse.masks import make_identity

    dpad = meta["dpad"]
    C = meta["C"]
    nbat = [meta["nbat_a"], meta["nbat_b"]]
    nslot = [meta["nslot_a"], meta["nslot_b"]]
    f16, f32, i16 = mybir.dt.float16, mybir.dt.float32, mybir.dt.int16
    Alu = mybir.AluOpType
    Act = mybir.ActivationFunctionType

    nc = bacc.Bacc("TRN2", target_bir_lowering=False, debug=False,
                   num_devices=NCORES)

    # --- I/O ---
    xt = nc.dram_tensor("xt", [F_IN, NPC], f32, kind="ExternalInput")
    wbd = nc.dram_tensor("wbd", [128, NBLK, dpad], f32, kind="ExternalInput")
    idx_in = [nc.dram_tensor("idxa", [nbat[0], 32, BATCH * 8], i16, kind="ExternalInput"),
              nc.dram_tensor("idxb", [nbat[1], 32, BATCH * 8], i16, kind="ExternalInput")]
    w_in = [nc.dram_tensor("wa", [128, nslot[0]], f32, kind="ExternalInput"),
            nc.dram_tensor("wb", [128, nslot[1]], f32, kind="ExternalInput")]
    d_in = [nc.dram_tensor("da", [128, nslot[0]], f32, kind="ExternalInput"),
            nc.dram_tensor("db", [128, nslot[1]], f32, kind="ExternalInput")]
    wg_in = [nc.dram_tensor(f"wg{i}", [F_IN if i == 1 else UNITS, FW], f32,
                            kind="ExternalInput") for i in (1, 2, 3)]
    ws_in = [nc.dram_tensor(f"ws{i}", [F_IN if i == 1 else UNITS, FW], f32,
                            kind="ExternalInput") for i in (1, 2, 3)]
    bn_in = [nc.dram_tensor(f"bn{i}", [1, 5 * FW], f32, kind="ExternalInput")
             for i in (1, 2, 3)]
    iota_in = nc.dram_tensor("iota", [128, 128], f16, kind="ExternalInput")
    ones_in = nc.dram_tensor("ones", [1, 128], f32, kind="ExternalInput")
    out_ext = nc.dram_tensor("out", [NPC, NCLS], f32, kind="ExternalOutput")

    with tile.TileContext(nc) as tc:
        with (
            tc.tile_pool(name="pers", bufs=1) as pers,
            tc.tile_pool(name="stream", bufs=4) as stream,
            tc.tile_pool(name="spool", bufs=6) as spool,
            tc.tile_pool(name="gat", bufs=8) as gatp,
            tc.tile_pool(name="idxp", bufs=8) as idxp,
            tc.tile_pool(name="pa", bufs=2, space="PSUM") as pa,
            tc.tile_pool(name="pb", bufs=2, space="PSUM") as pb,
            tc.tile_pool(name="pt", bufs=1, space="PSUM") as pt,
            tc.tile_pool(name="ps2", bufs=1, space="PSUM") as ps2,
            tc.tile_pool(name="ptr", bufs=2, space="PSUM") as ptr,
            tc.tile_pool(name="dram", bufs=1, space="DRAM") as dram,
        ):
            # ---------- setup ----------
            iota_t = pers.tile([128, 128], f16, tag="iota", name="iota")
            nc.sync.dma_start(iota_t[:], iota_in[:])
            ones_t = pers.tile([1, 128], f32, tag="ones", name="ones")
            nc.sync.dma_start(ones_t[:], ones_in[:])
            ident = pers.tile([128, 128], f32, tag="ident", name="ident")
            make_identity(nc, ident[:])

            wt = [pers.tile([128, nslot[h]], f32, tag=f"w{h}", name=f"w{h}")
                  for h in (0, 1)]
            dt_ = [pers.tile([128, nslot[h]], f32, tag=f"d{h}", name=f"d{h}")
                   for h in (0, 1)]
            for h in (0, 1):
                nc.sync.dma_start(wt[h][:], w_in[h][:])
                nc.sync.dma_start(dt_[h][:], d_in[h][:])

            # degree -> dinv / dinvinv  [128, NBLK]
            wbd_t = pers.tile([128, NBLK, dpad], f32, tag="wbd", name="wbd")
            nc.sync.dma_start(wbd_t[:], wbd[:])
            epsc = pers.tile([128, 1], f32, tag="epsc", name="epsc")
            nc.vector.memset(epsc[:], 1e-30)
            epsr = pers.tile([1, 1], f32, tag="epsr", name="epsr")
            nc.vector.memset(epsr[:], EPS)
            deg = pers.tile([128, NBLK], f32, tag="deg", name="deg")
            nc.vector.tensor_reduce(deg[:], wbd_t[:], axis=mybir.AxisListType.X,
                                    op=Alu.add)
            sq = pers.tile([128, NBLK], f32, tag="sq", name="sq")
            nc.scalar.activation(sq[:], deg[:], Act.Sqrt, bias=epsc[:, 0:1])
            rec = pers.tile([128, NBLK], f32, tag="rec", name="rec")
            nc.vector.reciprocal(rec[:], sq[:])
            mask = pers.tile([128, NBLK], f32, tag="mask", name="mask")
            nc.vector.tensor_scalar(mask[:], deg[:], 0.0, None, Alu.is_gt)
            dinv = pers.tile([128, NBLK], f32, tag="dinv", name="dinv")
            nc.vector.tensor_tensor(out=dinv[:], in0=rec[:], in1=mask[:], op=Alu.mult)
            dinvinv = pers.tile([128, NBLK], f32, tag="dinvinv", name="dinvinv")
            nc.vector.tensor_tensor(out=dinvinv[:], in0=sq[:], in1=mask[:], op=Alu.mult)

            # BN folding per layer
            grep, c0row, wgp, wsp = [None] * 3, [None] * 3, [None] * 3, [None] * 3

            def fold_bn(li):
                fi = F_IN if li == 0 else UNITS
                bnt = pers.tile([1, 5 * FW], f32, tag=f"bn{li}", name=f"bn{li}")
                nc.sync.dma_start(bnt[:], bn_in[li][:])
                sqv = pers.tile([1, FW], f32, tag=f"sqv{li}", name=f"sqv{li}")
                nc.scalar.activation(sqv[:], bnt[:, 4 * FW : 5 * FW], Act.Sqrt,
                                     bias=epsr[:, 0:1])
                recv = pers.tile([1, FW], f32, tag=f"recv{li}", name=f"recv{li}")
                nc.vector.reciprocal(recv[:], sqv[:])
                gr = pers.tile([1, FW], f32, tag=f"grow{li}", name=f"grow{li}")
                nc.vector.tensor_tensor(out=gr[:], in0=recv[:],
                                        in1=bnt[:, FW : 2 * FW], op=Alu.mult)
                c0 = pers.tile([1, FW], f32, tag=f"c0{li}", name=f"c0{li}")
                t1 = pers.tile([1, FW], f32, tag=f"t1{li}", name=f"t1{li}")
                nc.vector.tensor_tensor(out=t1[:], in0=bnt[:, 0:FW],
                                        in1=bnt[:, 3 * FW : 4 * FW], op=Alu.subtract)
                nc.vector.tensor_tensor(out=t1[:], in0=t1[:], in1=gr[:], op=Alu.mult)
                nc.vector.tensor_tensor(out=c0[:], in0=t1[:],
                                        in1=bnt[:, 2 * FW : 3 * FW], op=Alu.add)
                c0row[li] = c0
                gp_ps = pt.tile([128, FW], f32, tag="ps_t", name="gp_ps")
                nc.tensor.matmul(gp_ps[:], ones_t[:], gr[:], start=True, stop=True)
                gp = pers.tile([128, FW], f32, tag=f"grep{li}", name=f"grep{li}")
                nc.vector.tensor_copy(out=gp[:], in_=gp_ps[:])
                grep[li] = gp

                ntile = fi // 128
                wgl, wsl = [], []
                for k in range(ntile):
                    for (dst_list, src_dram, nm) in ((wgl, wg_in[li], "wg"),
                                                    (wsl, ws_in[li], "ws")):
                        raw = stream.tile([128, FW], f32, tag="wraw", name="wraw")
                        nc.sync.dma_start(raw[:], src_dram[k * 128 : (k + 1) * 128, :])
                        wp = pers.tile([128, FW], f32, tag=f"{nm}p{li}_{k}",
                                       name=f"{nm}p{li}_{k}")
                        nc.vector.tensor_tensor(out=wp[:], in0=raw[:], in1=gp[:],
                                                op=Alu.mult)
                        dst_list.append(wp)
                wgp[li] = wgl
                wsp[li] = wsl

            fold_bn(0)

            # persistent big buffers
            accA = pers.tile([128, NBLK * FW], f32, tag="accA", name="accA")
            selfb = pers.tile([128, NBLK * FW], f32, tag="selfb", name="selfb")
            tbls = pers.tile([128, NBLK, FW], f16, tag="tbls", name="tbls")
            hT = [pers.tile([128, NPC], f32, tag="hT1", name="hT1"),
                  pers.tile([128, NPC], f32, tag="hT2", name="hT2")]

            # ---------- layers ----------
            t_in = {}
            t_out = {}

            def emit_table_slice(li, j, lhs):
                # lhs: list of [128, 128] APs (K-tiles of h^T or x^T for block j)
                ktiles = len(lhs)
                ps_t = pt.tile([128, FW], f32, tag="ps_t", name="ps_t")
                ps_s = ps2.tile([128, FW], f32, tag="ps_s", name="ps_s")
                for k in range(ktiles):
                    nc.tensor.matmul(ps_t[:], lhs[k], wgp[li][k][:],
                                     start=(k == 0), stop=(k == ktiles - 1))
                    nc.tensor.matmul(ps_s[:], lhs[k], wsp[li][k][:],
                                     start=(k == 0), stop=False)
                nc.tensor.matmul(ps_s[:], ones_t[:], c0row[li][:],
                                 start=False, stop=True)
                nc.scalar.activation(tbls[:, j, :], ps_t[:], Act.Copy,
                                     scale=dinv[:, j : j + 1])
                nc.scalar.activation(selfb[:, j * FW : (j + 1) * FW], ps_s[:],
                                     Act.Copy, scale=dinvinv[:, j : j + 1])

            def emit_ag(li, h):
                # stage the finished table half to DRAM and AllGather it
                ti = dram.tile([HA, FW], f16, tag=f"tin{li}{h}",
                               name=f"tin{li}{h}")
                to = dram.tile([NCORES * HA, FW], f16, tag=f"tout{li}{h}",
                               name=f"tout{li}{h}")
                t_in[(li, h)] = ti
                t_out[(li, h)] = to
                jlo = 0 if h == 0 else 17
                nc.sync.dma_start(
                    ti[:].rearrange("(j p) f -> p j f", p=128),
                    tbls[:, jlo : jlo + 32, :])
                nc.gpsimd.collective_compute(
                    "AllGather", Alu.bypass,
                    replica_groups=[list(range(NCORES))],
                    ins=[ti.opt()], outs=[to.opt()])

            # layer-1 table from x^T (streamed from DRAM)
            for j in range(NBLK):
                lhs = []
                for k in range(2):
                    xa = stream.tile([128, 128], f32, tag="xs", name="xs")
                    nc.sync.dma_start(
                        xa[:], xt[k * 128 : (k + 1) * 128,
                                  j * 128 : (j + 1) * 128])
                    lhs.append(xa[:])
                emit_table_slice(0, j, lhs)
                if j == 31:
                    emit_ag(0, 0)
                if j == NBLK - 1:
                    emit_ag(0, 1)
            fold_bn(1)
            fold_bn(2)

            for li in range(3):
                relu = li < 2
                for h in (0, 1):
                    gts = []
                    for b in range(nbat[h]):
                        it = idxp.tile([128, BATCH * 8], i16, tag="idx", name="idx")
                        nc.sync.dma_start(it[0:32, :], idx_in[h][b, :, :])
                        gt = gatp.tile([128, BATCH, FW], f16, tag="gat", name="gat")
                        nc.gpsimd.dma_gather(gt[:], t_out[(li, h)][:], it[:],
                                             BATCH * 128, BATCH * 128, FW,
                                             single_packet=True)
                        gts.append(gt)
                    c = 0
                    for j in range(NBLK):
                        nchunks = int(C[j, h])
                        if h == 0:
                            psm = pa.tile([128, FW], f32, tag="psa", name="psa")
                        else:
                            psm = pb.tile([128, FW], f32, tag="psb", name="psb")
                            nc.tensor.matmul(psm[:], ident[:],
                                             accA[:, j * FW : (j + 1) * FW],
                                             start=True, stop=False)
                            nc.tensor.matmul(psm[:], ident[:],
                                             selfb[:, j * FW : (j + 1) * FW],
                                             start=False, stop=False)
                        for k in range(nchunks):
                            S = spool.tile([128, 128], f16, tag="S", name="S")
                            nc.vector.tensor_scalar(
                                S[:], iota_t[:],
                                dt_[h][:, c : c + 1], wt[h][:, c : c + 1],
                                Alu.is_equal, Alu.mult)
                            nc.tensor.matmul(
                                psm[:], S[:], gts[c // BATCH][:, c % BATCH, :],
                                start=(h == 0 and k == 0),
                                stop=(k == nchunks - 1))
                            c += 1
                        if h == 0:
                            nc.scalar.activation(accA[:, j * FW : (j + 1) * FW],
                                                 psm[:], Act.Copy)
                        else:
                            hblk = stream.tile([128, FW], f32, tag="hblk",
                                               name="hblk")
                            nc.scalar.activation(
                                hblk[:], psm[:],
                                Act.Relu if relu else Act.Copy,
                                scale=dinv[:, j : j + 1])
                            if li < 2:
                                tr = ptr.tile([128, 128], f32, tag="tr", name="tr")
                                nc.tensor.transpose(tr[:], hblk[:], ident[:])
                                nc.scalar.copy(hT[li][:, j * 128 : (j + 1) * 128],
                                               tr[:])
                                # next layer's table slice rides phase B
                                emit_table_slice(
                                    li + 1, j,
                                    [hT[li][:, j * 128 : (j + 1) * 128]])
                                if j == 31:
                                    emit_ag(li + 1, 0)
                                if j == NBLK - 1:
                                    emit_ag(li + 1, 1)
                            else:
                                nc.sync.dma_start(
                                    out_ext[j * 128 : (j + 1) * 128, :],
                                    hblk[:, 0:NCLS])

    nc.compile()
    return nc


_CACHE = {}


def _install_ntff_shim():
    try:
        import antenv
        if "antenv.axon_hooks" in sys.modules:
            return
        mod = types.ModuleType("antenv.axon_hooks")
        hook = [None]
        mod.set_axon_ntff_profile_hook = lambda h: hook.__setitem__(0, h)
        mod.get_axon_ntff_profile_hook = lambda: hook[0]
        sys.modules["antenv.axon_hooks"] = mod
        antenv.axon_hooks = mod
        from trn_agent_boot.trn_boot import _ntff_profile_via_ctypes
        mod.set_axon_ntff_profile_hook(
            _ntff_profile_via_ctypes("/opt/axon/libaxon_pjrt.so"))
    except Exception:
        pass


def kernel(trace=False, **inputs):
    from concourse import bass_utils

    meta, percore, shared = _host_prep(inputs)
    key = (meta["dpad"], meta["nbat_a"], meta["nbat_b"], meta["C"].tobytes())
    if key not in _CACHE:
        _CACHE[key] = _build(meta)
    nc = _CACHE[key]

    in_maps = []
    for c in range(NCORES):
        m = dict(shared)
        m["xt"] = percore["xt"][c]
        m["wbd"] = percore["wbd"][c]
        m["idxa"] = percore["idxa"][c]
        m["idxb"] = percore["idxb"][c]
        m["wa"] = percore["wa"][c]
        m["wb"] = percore["wb"][c]
        m["da"] = percore["da"][c]
        m["db"] = percore["db"][c]
        in_maps.append(m)

    if trace:
        _install_ntff_shim()
    res = bass_utils.run_bass_kernel_spmd(
        nc, in_maps, core_ids=list(range(NCORES)), trace=trace)
    full = np.concatenate([res.results[c]["out"] for c in range(NCORES)], axis=0)
    out = full[:N].astype(np.float32)
    if trace:
        kernel.last_exec_time_ns = res.exec_time_ns
    return out
